# revision 14
# baseline (speedup 1.0000x reference)
"""Trainium2 Bass kernel for Box3dEncoder (nn_Box3dEncoder_75453985456565).

Contract: kernel(**inputs) takes the FULL inputs
    corners3d        [4, 16, 8, 3] f32
    neck_voxel_sizes [4, 3]        f32
and returns the FULL output [4*32768, 2] f32.

Strategy (per the voxel-sharding hint): the 64x64x8 grid's first axis (i)
is sharded 8 ways -> each core owns 512 XY cells (8 i-rows x 64 j) x 8 k
levels x 4 batches. Per-box scalars (edge params, z-overlap/volume factors,
heading encodings) are tiny host-side prep, replicated to all cores; all
O(V*N) work runs on-device:

  stage 2 (per core): branch-free polygon-rect clipped area via Green's
    theorem - per edge, integrate clamp(x(t),x0,x1)-x0 over the t-range
    where y(t) is inside [y0,y1], with trapezoid splits at the x-crossings.
    Layout: 128 partitions = (2 batches x 16 boxes x 4 edges), free = 512
    cells. The per-edge factor dy/2 is folded into the PE reduction matmul
    that sums the 4 edges and transposes to cells-major.
  stage 3: rho = area * (zov/C) is a per-(k,n) positive rescale of IoU that
    preserves the argmax (iou = inter/(C-inter) is monotone in inter), so
    argmax over boxes runs division-free; first-occurrence tie-break via
    eq*(BIG-n) + reduce_max; one-hot selection of C, sin, cos; threshold
    mask; interleaved (sin,cos) store.
"""
import numpy as np

B, N, K = 4, 16, 8
CUBE = (64, 64, 8)
LOW = (-32, -32, -4)
NCORES = 8
NI = CUBE[0] // NCORES          # 8 i-rows per core
NJ = CUBE[1]                    # 64
NCELL = NI * NJ                 # 512 cells per core
NCHUNK = NCELL // 128           # 4
V = CUBE[0] * CUBE[1] * CUBE[2]
BIG = 1024.0

_COMPILED = None


def _host_prep(corners3d, neck_voxel_sizes):
    c = np.asarray(corners3d, np.float32)
    vs = np.asarray(neck_voxel_sizes, np.float32)[0]
    vox_vol = np.float32(vs[0]) * np.float32(vs[1]) * np.float32(vs[2])

    poly = c[:, :, :4, :2]                     # [B,N,4,2]
    nxt = np.roll(poly, -1, axis=2)
    xa, ya = poly[..., 0], poly[..., 1]        # [B,N,4]
    xb, yb = nxt[..., 0], nxt[..., 1]
    dx, dy = xb - xa, yb - ya
    with np.errstate(divide='ignore'):
        inv_dx = np.where(np.abs(dx) < 1e-12, np.float32(0),
                          np.float32(1) / np.where(dx == 0, np.float32(1), dx))
        inv_dy = np.where(np.abs(dy) < 1e-12, np.float32(0),
                          np.float32(1) / np.where(dy == 0, np.float32(1), dy))

    zb0 = c[:, :, :, 2].min(axis=2)
    zb1 = c[:, :, :, 2].max(axis=2)
    quad_area = 0.5 * np.abs((xa * yb - xb * ya).sum(axis=2))
    box_vol = quad_area * (zb1 - zb0)
    C = (vox_vol + box_vol + np.float32(1e-9)).astype(np.float32)   # [B,N]
    invC = (np.float32(1) / C).astype(np.float32)

    kk = np.arange(K, dtype=np.float32) + LOW[2]
    z0 = kk * vs[2]
    z1 = (kk + 1) * vs[2]
    zov = np.maximum(np.minimum(z1[None, :, None], zb1[:, None, :])
                     - np.maximum(z0[None, :, None], zb0[:, None, :]),
                     np.float32(0))                                  # [B,K,N]
    zrho = (zov * invC[:, None, :]).astype(np.float32)

    d = c[:, :, 0, :2] - c[:, :, 3, :2]
    h = np.sqrt(d[..., 0] ** 2 + d[..., 1] ** 2)
    hs = np.where(h == 0, np.float32(1), h)
    sin = np.where(h > 0, d[..., 1] / hs, np.float32(0)).astype(np.float32)
    cos = np.where(h > 0, d[..., 0] / hs, np.float32(1)).astype(np.float32)

    # --- stage-2 per-partition columns, p = b_lo*64 + n*4 + e, per h-iter ---
    def colpack(a):    # [B,N,4] -> [2h][128]
        return a.reshape(2, 2, N, 4).reshape(2, 128)
    DX_EPS = np.float32(1e-4)
    vert = np.abs(dx) < DX_EPS
    w1 = np.where(vert, np.float32(0), dy * inv_dx * np.float32(0.5))
    w2 = np.where(vert, dy, np.float32(0))
    cols = np.zeros((2, 128, 8), np.float32)
    cols[:, :, 0] = colpack(inv_dy)
    cols[:, :, 1] = colpack(-ya * inv_dy)
    cols[:, :, 2] = colpack((vs[1] - ya) * inv_dy)
    cols[:, :, 3] = colpack(dx)
    cols[:, :, 4] = colpack(xa)
    cols[:, :, 5] = colpack(w1)
    cols[:, :, 6] = colpack(w2)
    cols = np.ascontiguousarray(cols.transpose(1, 0, 2))       # [128,2,8]

    # edge-reduction weights with zrho folded in:
    # rw2[p=(b_lo,n,e), h, (b_lo',k,n')] = (b_lo'==b_lo & n'==n) * dy/2 * zrho[b,k,n]
    rw = np.zeros((128, 2, 2, K, N), np.float32)
    for h in range(2):
        for p in range(128):
            b_lo, n = p // 64, (p % 64) // 4
            rw[p, h, b_lo, :, n] = zrho[2 * h + b_lo, :, n]
    rw = np.ascontiguousarray(rw.reshape(128, 2, 2 * K * N))   # [128,2,256]

    # selection matmul weights: w4[b][(k,n), (k',q)] = (k==k') * w_q[b,n]
    # q in {C, sin, cos}
    w4 = np.zeros((128, B, K, 3), np.float32)
    for p in range(128):
        kq, n = p // N, p % N
        w4[p, :, kq, 0] = C[:, n]
        w4[p, :, kq, 1] = sin[:, n]
        w4[p, :, kq, 2] = cos[:, n]
    w4 = np.ascontiguousarray(w4.reshape(128, B * K * 3))      # [128,96]

    # --- cells-major broadcast constants (partition-replicated by host) ---
    kbig = BIG - np.arange(N, dtype=np.float32)                # [16]
    jj = np.arange(NJ, dtype=np.float32) + LOW[1]
    y0 = (jj * vs[1]).astype(np.float32)                       # [64]
    halfvol = np.float32(0.5) * vox_vol

    consts = []
    for m in range(NCORES):
        ii = np.arange(NI, dtype=np.float32) + (m * NI + LOW[0])
        x0 = (ii * vs[0]).astype(np.float32)                   # [8]
        row = np.concatenate([kbig, y0, x0,
                              [halfvol, vs[0], -vs[0], 2 * vs[0]]]).astype(np.float32)
        consts.append(np.broadcast_to(row, (128, row.size)).copy())
    ident = np.eye(128, dtype=np.float32)
    return cols, rw, w4, ident, consts


def _build(stages='all'):
    import concourse.bass as bass
    import concourse.tile as tile
    from concourse import bacc, mybir

    f32 = mybir.dt.float32
    ALU = mybir.AluOpType
    ACT = mybir.ActivationFunctionType

    CW = 16 + 64 + 8 + 4
    OFF_KBIG, OFF_Y0, OFF_X0, OFF_MISC = 0, 16, 80, 88

    nc = bacc.Bacc("TRN2", target_bir_lowering=False, debug=False,
                   num_devices=NCORES)
    d_consts = nc.dram_tensor("consts", [128, CW], f32, kind="ExternalInput")
    d_cols = nc.dram_tensor("cols", [128, 2, 8], f32, kind="ExternalInput")
    d_rw = nc.dram_tensor("rw", [128, 2, 256], f32, kind="ExternalInput")
    d_w4 = nc.dram_tensor("w4", [128, B * K * 3], f32, kind="ExternalInput")
    d_ident = nc.dram_tensor("ident", [128, 128], f32, kind="ExternalInput")
    d_out = nc.dram_tensor("out", [B, NCELL * K, 2], f32, kind="ExternalOutput")

    with tile.TileContext(nc) as tc:
        with (
            tc.tile_pool(name="const", bufs=1) as cpool,
            tc.tile_pool(name="small", bufs=4) as spool,
            tc.tile_pool(name="work", bufs=3) as wpool,
            tc.tile_pool(name="edge", bufs=2) as epool,
            tc.tile_pool(name="st3", bufs=4) as tpool,
            tc.tile_pool(name="outp", bufs=4) as opool,
            tc.tile_pool(name="psum", bufs=1, space=bass.MemorySpace.PSUM) as ppool,
            tc.tile_pool(name="psum2", bufs=2, space=bass.MemorySpace.PSUM) as ppool2,
        ):
            tco = cpool.tile([128, CW], f32, tag="consts")
            nc.sync.dma_start(tco[:], d_consts[:])
            tcols = cpool.tile([128, 2, 8], f32, tag="cols")
            nc.scalar.dma_start(tcols[:], d_cols[:])
            trw = cpool.tile([128, 2, 256], f32, tag="rw")
            nc.sync.dma_start(trw[:], d_rw[:])
            tw4 = cpool.tile([128, B * K * 3], f32, tag="w4")
            nc.scalar.dma_start(tw4[:], d_w4[:])
            ident = cpool.tile([128, 128], f32, tag="ident")
            nc.sync.dma_start(ident[:], d_ident[:])

            kbig_bc = tco[:, OFF_KBIG:OFF_KBIG + 16]
            y0_bc = tco[:, OFF_Y0:OFF_Y0 + 64]
            x0_bc = tco[:, OFF_X0:OFF_X0 + 8]
            halfvol_col = tco[:, OFF_MISC:OFF_MISC + 1]
            vs0_col = tco[:, OFF_MISC + 1:OFF_MISC + 2]
            nvs0_col = tco[:, OFF_MISC + 2:OFF_MISC + 3]
            vs0x2_col = tco[:, OFF_MISC + 3:OFF_MISC + 4]

            def bj(ap):   # [128,64] j-tile -> broadcast over i: [128,8,64]
                return ap[:, None, :].broadcast_to([128, NI, NJ])

            def bi(ap):   # [128,8] i-tile -> broadcast over j: [128,8,64]
                return ap[:, :, None].broadcast_to([128, NI, NJ])

            rho_ps = []
            for c in range(NCHUNK):
                rp = ppool.tile([128, B * K * N], f32, tag=f"rho{c}")
                rho_ps.append(rp)
            for h in range(2 if stages != 'none' else 0):
                col = lambda q: tcols[:, h, q:q + 1]
                # j-only quantities [128, 64]
                ty0 = spool.tile([128, NJ], f32, tag="ty0")
                nc.scalar.activation(ty0[:], y0_bc, ACT.Identity,
                                     bias=col(1), scale=col(0))
                ty1 = spool.tile([128, NJ], f32, tag="ty1")
                nc.scalar.activation(ty1[:], y0_bc, ACT.Identity,
                                     bias=col(2), scale=col(0))
                u0 = spool.tile([128, NJ], f32, tag="u0")
                nc.vector.tensor_scalar(u0[:], ty0[:], 0.0, 1.0, ALU.max, ALU.min)
                u1 = spool.tile([128, NJ], f32, tag="u1")
                nc.vector.tensor_scalar(u1[:], ty1[:], 0.0, 1.0, ALU.max, ALU.min)
                lo = spool.tile([128, NJ], f32, tag="lo")
                nc.vector.tensor_tensor(lo[:], u0[:], u1[:], ALU.min)
                hi = spool.tile([128, NJ], f32, tag="hi")
                nc.vector.tensor_tensor(hi[:], u0[:], u1[:], ALU.max)
                hilo = spool.tile([128, NJ], f32, tag="hilo")
                nc.vector.tensor_tensor(hilo[:], hi[:], lo[:], ALU.subtract)
                # i-only quantities [128, 8]
                x0mxa = spool.tile([128, NI], f32, tag="x0mxa")
                nc.vector.tensor_single_scalar(x0mxa[:], x0_bc, col(4),
                                               ALU.subtract)
                # vertical-edge fallback: Fv = clamp(xa - x0, 0, vs0), * w2
                fvw = spool.tile([128, NI], f32, tag="fvw")
                nc.vector.tensor_scalar(fvw[:], x0mxa[:], -1.0, 0.0,
                                        ALU.mult, ALU.max)
                nc.vector.tensor_single_scalar(fvw[:], fvw[:], vs0_col, ALU.min)
                nc.vector.tensor_single_scalar(fvw[:], fvw[:], col(6), ALU.mult)

                def full(tag):
                    t = wpool.tile([128, NI, NJ], f32, tag=tag)
                    return t

                # g at t=lo and t=hi:  g = dx*t - (x0 - xa)
                glo = full("glo")
                nc.vector.scalar_tensor_tensor(glo[:], bj(lo[:]), col(3),
                                               bi(x0mxa[:]), ALU.mult,
                                               ALU.subtract)
                ghi = full("ghi")
                nc.vector.scalar_tensor_tensor(ghi[:], bj(hi[:]), col(3),
                                               bi(x0mxa[:]), ALU.mult,
                                               ALU.subtract)
                # H(u) = 0.5*clamp(u,0,c)^2 + c*relu(u-c); w1 carries the 0.5
                clo = full("clo")
                nc.vector.tensor_scalar(clo[:], glo[:], 0.0, vs0_col,
                                        ALU.max, ALU.min)
                chi = full("chi")
                nc.gpsimd.tensor_scalar(chi[:], ghi[:], 0.0, vs0_col,
                                        ALU.max, ALU.min)
                sqlo = full("sqlo")
                nc.scalar.activation(sqlo[:], clo[:], ACT.Square)
                sqhi = full("sqhi")
                nc.scalar.activation(sqhi[:], chi[:], ACT.Square)
                rlo = full("rlo")
                nc.scalar.activation(rlo[:], glo[:], ACT.Relu, bias=nvs0_col)
                rhi = full("rhi")
                nc.scalar.activation(rhi[:], ghi[:], ACT.Relu, bias=nvs0_col)
                e1 = full("e1")
                nc.vector.tensor_tensor(e1[:], sqhi[:], sqlo[:], ALU.subtract)
                e2 = full("e2")
                nc.gpsimd.tensor_tensor(e2[:], rhi[:], rlo[:], ALU.subtract)
                s = full("s")
                nc.vector.scalar_tensor_tensor(s[:], e2[:], vs0x2_col, e1[:],
                                               ALU.mult, ALU.add)
                t2w = full("t2w")
                nc.gpsimd.tensor_tensor(t2w[:], bi(fvw[:]), bj(hilo[:]),
                                        ALU.mult)
                iedge = epool.tile([128, NCELL], f32, tag="iedge")
                nc.vector.scalar_tensor_tensor(
                    iedge[:].rearrange("p (i j) -> p i j", j=NJ),
                    s[:], col(5), t2w[:], ALU.mult, ALU.add)

                for cch in range(NCHUNK):
                    nc.tensor.matmul(
                        rho_ps[cch][:, h * 256:(h + 1) * 256],
                        iedge[:, cch * 128:(cch + 1) * 128],
                        trw[:, h, :], start=True, stop=True)

            # ---- stage 3, cells-major, per 128-cell chunk ----
            for cch in range(NCHUNK if stages == 'all' else 0):
                rho3 = rho_ps[cch][:].rearrange("p (g n) -> p g n", n=N)
                maxrho = tpool.tile([128, B * K], f32, tag="maxrho")
                nc.vector.reduce_max(maxrho[:], rho3, axis=mybir.AxisListType.X)
                mx_bc = maxrho[:][:, :, None].broadcast_to([128, B * K, N])
                eq = tpool.tile([128, B * K * N], f32, tag="eq")
                nc.vector.tensor_tensor(
                    eq[:].rearrange("p (g n) -> p g n", n=N), rho3, mx_bc,
                    ALU.is_equal)
                cand = tpool.tile([128, B * K * N], f32, tag="cand")
                kb_bc = kbig_bc[:, None, :].broadcast_to([128, B * K, N])
                nc.gpsimd.tensor_tensor(
                    cand[:].rearrange("p (g n) -> p g n", n=N),
                    eq[:].rearrange("p (g n) -> p g n", n=N), kb_bc, ALU.mult)
                cand3 = cand[:].rearrange("p (g n) -> p g n", n=N)
                idxv = tpool.tile([128, B * K], f32, tag="idxv")
                nc.vector.reduce_max(idxv[:], cand3, axis=mybir.AxisListType.X)
                onehot = tpool.tile([128, B * K * N], f32, tag="onehot")
                nc.vector.tensor_tensor(
                    onehot[:].rearrange("p (g n) -> p g n", n=N), cand3,
                    idxv[:][:, :, None].broadcast_to([128, B * K, N]),
                    ALU.is_equal)

                # selection sums via PE: transpose onehot per b, then matmul
                # against w4 -> SEL[cell, (k, {C,sin,cos})]
                oh_t = ppool2.tile([128, 4 * 128], f32, tag="oht")
                sel_ps = ppool2.tile([128, B * K * 3], f32, tag="selps")
                for b in range(B):
                    nc.tensor.transpose(
                        oh_t[:, b * 128:(b + 1) * 128],
                        onehot[:, b * 128:(b + 1) * 128], ident[:])
                    ohs = tpool.tile([128, 128], f32, tag="ohs")
                    nc.scalar.copy(ohs[:], oh_t[:, b * 128:(b + 1) * 128])
                    nc.tensor.matmul(
                        sel_ps[:, b * K * 3:(b + 1) * K * 3],
                        ohs[:], tw4[:, b * K * 3:(b + 1) * K * 3],
                        start=True, stop=True)

                sel3 = sel_ps[:].rearrange("p (b k q) -> p b k q", k=K, q=3)
                mx3 = maxrho[:].rearrange("p (b k) -> p b k", k=K)
                intersel = tpool.tile([128, B * K], f32, tag="intersel")
                nc.vector.tensor_tensor(
                    intersel[:].rearrange("p (b k) -> p b k", k=K),
                    mx3, sel3[:, :, :, 0], ALU.mult)
                mask = tpool.tile([128, B * K], f32, tag="mask")
                nc.vector.tensor_single_scalar(mask[:], intersel[:],
                                               halfvol_col, ALU.is_gt)
                outt = opool.tile([128, B, K, 2], f32, tag="outt")
                nc.vector.tensor_tensor(
                    outt[:, :, :, 0], sel3[:, :, :, 1],
                    mask[:].rearrange("p (b k) -> p b k", k=K), ALU.mult)
                nc.vector.tensor_tensor(
                    outt[:, :, :, 1], sel3[:, :, :, 2],
                    mask[:].rearrange("p (b k) -> p b k", k=K), ALU.mult)
                dma_eng = nc.sync if cch % 2 == 0 else nc.scalar
                dma_eng.dma_start(
                    d_out[:, cch * 128 * K:(cch + 1) * 128 * K, :]
                         .rearrange("b (p k) e -> p b k e", k=K),
                    outt[:])

    if stages != 'all':
        with tile.TileContext(nc) as tc2:
            with tc2.tile_pool(name="fin", bufs=1) as fpool:
                z = fpool.tile([128, 64], f32, tag="z")
                nc.gpsimd.memset(z[:], 0.0)
                nc.gpsimd.dma_start(
                    d_out[:, 0:1024, :].rearrange("b (p k) e -> p b k e", k=K),
                    z[:].rearrange("p (b k e) -> p b k e", k=K, e=2))
    nc.compile()
    return nc


def kernel(corners3d, neck_voxel_sizes):
    global _COMPILED
    from concourse.bass_utils import run_bass_kernel_spmd

    cols, rw, w4, ident, consts = _host_prep(corners3d, neck_voxel_sizes)
    if _COMPILED is None:
        _COMPILED = _build()
    nc = _COMPILED
    in_maps = [{"consts": consts[m], "cols": cols, "rw": rw, "w4": w4,
                "ident": ident} for m in range(NCORES)]
    res = run_bass_kernel_spmd(nc, in_maps, list(range(NCORES)))
    out = np.zeros((B, V, 2), np.float32)
    for m in range(NCORES):
        blk = res.results[m]["out"]          # [B, 4096, 2]
        out[:, m * NCELL * K:(m + 1) * NCELL * K, :] = blk
    return out.reshape(B * V, 2)


# revision 22
# speedup vs baseline: 1.0517x; 1.0517x over previous
"""Trainium2 Bass kernel for Box3dEncoder (nn_Box3dEncoder_75453985456565).

Contract: kernel(**inputs) takes the FULL inputs
    corners3d        [4, 16, 8, 3] f32
    neck_voxel_sizes [4, 3]        f32
and returns the FULL output [4*32768, 2] f32.

Strategy (per the voxel-sharding hint): the 64x64x8 grid's first axis (i)
is sharded 8 ways -> each core owns 512 XY cells (8 i-rows x 64 j) x 8 k
levels x 4 batches. Per-box scalars (edge params, z-overlap/volume factors,
heading encodings) are tiny host-side prep, replicated to all cores; all
O(V*N) work runs on-device:

  stage 2 (per core): branch-free polygon-rect clipped area via Green's
    theorem - per edge, integrate clamp(x(t),x0,x1)-x0 over the t-range
    where y(t) is inside [y0,y1], with trapezoid splits at the x-crossings.
    Layout: 128 partitions = (2 batches x 16 boxes x 4 edges), free = 512
    cells. The per-edge factor dy/2 is folded into the PE reduction matmul
    that sums the 4 edges and transposes to cells-major.
  stage 3: rho = area * (zov/C) is a per-(k,n) positive rescale of IoU that
    preserves the argmax (iou = inter/(C-inter) is monotone in inter), so
    argmax over boxes runs division-free; first-occurrence tie-break via
    eq*(BIG-n) + reduce_max; one-hot selection of C, sin, cos; threshold
    mask; interleaved (sin,cos) store.
"""
import numpy as np

B, N, K = 4, 16, 8
CUBE = (64, 64, 8)
LOW = (-32, -32, -4)
NCORES = 8
NI = CUBE[0] // NCORES          # 8 i-rows per core
NJ = CUBE[1]                    # 64
NCELL = NI * NJ                 # 512 cells per core
NCHUNK = NCELL // 128           # 4
V = CUBE[0] * CUBE[1] * CUBE[2]
BIG = 1024.0

_COMPILED = None


def _host_prep(corners3d, neck_voxel_sizes):
    c = np.asarray(corners3d, np.float32)
    vs = np.asarray(neck_voxel_sizes, np.float32)[0]
    vox_vol = np.float32(vs[0]) * np.float32(vs[1]) * np.float32(vs[2])

    poly = c[:, :, :4, :2]                     # [B,N,4,2]
    nxt = np.roll(poly, -1, axis=2)
    xa, ya = poly[..., 0], poly[..., 1]        # [B,N,4]
    xb, yb = nxt[..., 0], nxt[..., 1]
    dx, dy = xb - xa, yb - ya
    with np.errstate(divide='ignore'):
        inv_dx = np.where(np.abs(dx) < 1e-12, np.float32(0),
                          np.float32(1) / np.where(dx == 0, np.float32(1), dx))
        inv_dy = np.where(np.abs(dy) < 1e-12, np.float32(0),
                          np.float32(1) / np.where(dy == 0, np.float32(1), dy))

    zb0 = c[:, :, :, 2].min(axis=2)
    zb1 = c[:, :, :, 2].max(axis=2)
    quad_area = 0.5 * np.abs((xa * yb - xb * ya).sum(axis=2))
    box_vol = quad_area * (zb1 - zb0)
    C = (vox_vol + box_vol + np.float32(1e-9)).astype(np.float32)   # [B,N]
    invC = (np.float32(1) / C).astype(np.float32)

    kk = np.arange(K, dtype=np.float32) + LOW[2]
    z0 = kk * vs[2]
    z1 = (kk + 1) * vs[2]
    zov = np.maximum(np.minimum(z1[None, :, None], zb1[:, None, :])
                     - np.maximum(z0[None, :, None], zb0[:, None, :]),
                     np.float32(0))                                  # [B,K,N]
    zrho = (zov * invC[:, None, :]).astype(np.float32)

    d = c[:, :, 0, :2] - c[:, :, 3, :2]
    h = np.sqrt(d[..., 0] ** 2 + d[..., 1] ** 2)
    hs = np.where(h == 0, np.float32(1), h)
    sin = np.where(h > 0, d[..., 1] / hs, np.float32(0)).astype(np.float32)
    cos = np.where(h > 0, d[..., 0] / hs, np.float32(1)).astype(np.float32)

    # --- stage-2 per-partition columns, p = b_lo*64 + n*4 + e, per h-iter ---
    def colpack(a):    # [B,N,4] -> [2h][128]
        return a.reshape(2, 2, N, 4).reshape(2, 128)
    DX_EPS = np.float32(1e-4)
    vert = np.abs(dx) < DX_EPS
    w1 = np.where(vert, np.float32(0), dy * inv_dx * np.float32(0.5))
    w2 = np.where(vert, dy, np.float32(0))
    cols = np.zeros((2, 128, 8), np.float32)
    cols[:, :, 0] = colpack(inv_dy)
    cols[:, :, 1] = colpack(-ya * inv_dy)
    cols[:, :, 2] = colpack((vs[1] - ya) * inv_dy)
    cols[:, :, 3] = colpack(dx)
    cols[:, :, 4] = colpack(xa)
    cols[:, :, 5] = colpack(w1)
    cols[:, :, 6] = colpack(w2)
    cols = np.ascontiguousarray(cols.transpose(1, 0, 2))       # [128,2,8]

    # edge-reduction weights with zrho folded in:
    # rw2[p=(b_lo,n,e), h, (b_lo',k,n')] = (b_lo'==b_lo & n'==n) * dy/2 * zrho[b,k,n]
    rw = np.zeros((128, 2, 2, K, N), np.float32)
    for h in range(2):
        for p in range(128):
            b_lo, n = p // 64, (p % 64) // 4
            rw[p, h, b_lo, :, n] = zrho[2 * h + b_lo, :, n]
    rw = np.ascontiguousarray(rw.reshape(128, 2, 2 * K * N))   # [128,2,256]

    # selection matmul weights: w4[b][(k,n), (k',q)] = (k==k') * w_q[b,n]
    # q in {C, sin, cos}
    w4 = np.zeros((128, B, K, 3), np.float32)
    for p in range(128):
        kq, n = p // N, p % N
        w4[p, :, kq, 0] = C[:, n]
        w4[p, :, kq, 1] = sin[:, n]
        w4[p, :, kq, 2] = cos[:, n]
    w4 = np.ascontiguousarray(w4.reshape(128, B * K * 3))      # [128,96]

    # --- cells-major broadcast constants (partition-replicated by host) ---
    kbig = BIG - np.arange(N, dtype=np.float32)                # [16]
    jj = np.arange(NJ, dtype=np.float32) + LOW[1]
    y0 = (jj * vs[1]).astype(np.float32)                       # [64]
    halfvol = np.float32(0.5) * vox_vol

    consts = []
    for m in range(NCORES):
        ii = np.arange(NI, dtype=np.float32) + (m * NI + LOW[0])
        x0 = (ii * vs[0]).astype(np.float32)                   # [8]
        row = np.concatenate([kbig, y0, x0,
                              [halfvol, vs[0], -vs[0], 2 * vs[0]]]).astype(np.float32)
        consts.append(np.broadcast_to(row, (128, row.size)).copy())
    ident = np.eye(128, dtype=np.float32)
    return cols, rw, w4, ident, consts


def _build(stages='all'):
    import concourse.bass as bass
    import concourse.tile as tile
    from concourse import bacc, mybir

    f32 = mybir.dt.float32
    ALU = mybir.AluOpType
    ACT = mybir.ActivationFunctionType

    CW = 16 + 64 + 8 + 4
    OFF_KBIG, OFF_Y0, OFF_X0, OFF_MISC = 0, 16, 80, 88

    nc = bacc.Bacc("TRN2", target_bir_lowering=False, debug=False,
                   num_devices=NCORES)
    d_consts = nc.dram_tensor("consts", [128, CW], f32, kind="ExternalInput")
    d_cols = nc.dram_tensor("cols", [128, 2, 8], f32, kind="ExternalInput")
    d_rw = nc.dram_tensor("rw", [128, 2, 256], f32, kind="ExternalInput")
    d_w4 = nc.dram_tensor("w4", [128, B * K * 3], f32, kind="ExternalInput")
    d_ident = nc.dram_tensor("ident", [128, 128], f32, kind="ExternalInput")
    d_out = nc.dram_tensor("out", [B, NCELL * K, 2], f32, kind="ExternalOutput")

    with tile.TileContext(nc) as tc:
        with (
            tc.tile_pool(name="const", bufs=1) as cpool,
            tc.tile_pool(name="small", bufs=4) as spool,
            tc.tile_pool(name="work", bufs=4) as wpool,
            tc.tile_pool(name="edge", bufs=3) as epool,
            tc.tile_pool(name="st3", bufs=6) as tpool,
            tc.tile_pool(name="outp", bufs=4) as opool,
            tc.tile_pool(name="psum", bufs=1, space=bass.MemorySpace.PSUM) as ppool,
            tc.tile_pool(name="psum2", bufs=2, space=bass.MemorySpace.PSUM) as ppool2,
        ):
            tco = cpool.tile([128, CW], f32, tag="consts")
            nc.sync.dma_start(tco[:], d_consts[:])
            tcols = cpool.tile([128, 2, 8], f32, tag="cols")
            nc.scalar.dma_start(tcols[:], d_cols[:])
            trw = cpool.tile([128, 2, 256], f32, tag="rw")
            nc.sync.dma_start(trw[:], d_rw[:])
            tw4 = cpool.tile([128, B * K * 3], f32, tag="w4")
            nc.scalar.dma_start(tw4[:], d_w4[:])
            ident = cpool.tile([128, 128], f32, tag="ident")
            nc.sync.dma_start(ident[:], d_ident[:])

            kbig_bc = tco[:, OFF_KBIG:OFF_KBIG + 16]
            y0_bc = tco[:, OFF_Y0:OFF_Y0 + 64]
            x0_bc = tco[:, OFF_X0:OFF_X0 + 8]
            halfvol_col = tco[:, OFF_MISC:OFF_MISC + 1]
            vs0_col = tco[:, OFF_MISC + 1:OFF_MISC + 2]
            nvs0_col = tco[:, OFF_MISC + 2:OFF_MISC + 3]
            vs0x2_col = tco[:, OFF_MISC + 3:OFF_MISC + 4]

            def bj(ap):   # [128,64] j-tile -> broadcast over i: [128,8,64]
                return ap[:, None, :].broadcast_to([128, NI, NJ])

            def bi(ap):   # [128,8] i-tile -> broadcast over j: [128,8,64]
                return ap[:, :, None].broadcast_to([128, NI, NJ])

            rho_ps = []
            for c in range(NCHUNK):
                rp = ppool.tile([128, B * K * N], f32, tag=f"rho{c}")
                rho_ps.append(rp)
            for h in range(2 if stages != 'none' else 0):
                col = lambda q: tcols[:, h, q:q + 1]
                # j-only quantities [128, 64]
                ty0 = spool.tile([128, NJ], f32, tag="ty0")
                nc.scalar.activation(ty0[:], y0_bc, ACT.Identity,
                                     bias=col(1), scale=col(0))
                ty1 = spool.tile([128, NJ], f32, tag="ty1")
                nc.scalar.activation(ty1[:], y0_bc, ACT.Identity,
                                     bias=col(2), scale=col(0))
                u0 = spool.tile([128, NJ], f32, tag="u0")
                nc.vector.tensor_scalar(u0[:], ty0[:], 0.0, 1.0, ALU.max, ALU.min)
                u1 = spool.tile([128, NJ], f32, tag="u1")
                nc.vector.tensor_scalar(u1[:], ty1[:], 0.0, 1.0, ALU.max, ALU.min)
                lo = spool.tile([128, NJ], f32, tag="lo")
                nc.vector.tensor_tensor(lo[:], u0[:], u1[:], ALU.min)
                hi = spool.tile([128, NJ], f32, tag="hi")
                nc.vector.tensor_tensor(hi[:], u0[:], u1[:], ALU.max)
                hilo = spool.tile([128, NJ], f32, tag="hilo")
                nc.vector.tensor_tensor(hilo[:], hi[:], lo[:], ALU.subtract)
                # i-only quantities [128, 8]
                x0mxa = spool.tile([128, NI], f32, tag="x0mxa")
                nc.vector.tensor_single_scalar(x0mxa[:], x0_bc, col(4),
                                               ALU.subtract)
                # vertical-edge fallback: Fv = clamp(xa - x0, 0, vs0), * w2
                fvw = spool.tile([128, NI], f32, tag="fvw")
                nc.vector.tensor_scalar(fvw[:], x0mxa[:], -1.0, 0.0,
                                        ALU.mult, ALU.max)
                nc.vector.tensor_single_scalar(fvw[:], fvw[:], vs0_col, ALU.min)
                nc.vector.tensor_single_scalar(fvw[:], fvw[:], col(6), ALU.mult)

                def full(tag):
                    t = wpool.tile([128, NI, NJ], f32, tag=tag)
                    return t

                # g at t=lo and t=hi:  g = dx*t - (x0 - xa)
                glo = full("glo")
                nc.vector.scalar_tensor_tensor(glo[:], bj(lo[:]), col(3),
                                               bi(x0mxa[:]), ALU.mult,
                                               ALU.subtract)
                ghi = full("ghi")
                nc.vector.scalar_tensor_tensor(ghi[:], bj(hi[:]), col(3),
                                               bi(x0mxa[:]), ALU.mult,
                                               ALU.subtract)
                # H(u) = 0.5*clamp(u,0,c)^2 + c*relu(u-c); w1 carries the 0.5
                clo = full("clo")
                nc.vector.tensor_scalar(clo[:], glo[:], 0.0, vs0_col,
                                        ALU.max, ALU.min)
                chi = full("chi")
                nc.gpsimd.tensor_scalar(chi[:], ghi[:], 0.0, vs0_col,
                                        ALU.max, ALU.min)
                sqlo = full("sqlo")
                nc.scalar.activation(sqlo[:], clo[:], ACT.Square)
                sqhi = full("sqhi")
                nc.scalar.activation(sqhi[:], chi[:], ACT.Square)
                rlo = full("rlo")
                nc.scalar.activation(rlo[:], glo[:], ACT.Relu, bias=nvs0_col)
                rhi = full("rhi")
                nc.scalar.activation(rhi[:], ghi[:], ACT.Relu, bias=nvs0_col)
                e1 = full("e1")
                nc.vector.tensor_tensor(e1[:], sqhi[:], sqlo[:], ALU.subtract)
                e2 = full("e2")
                nc.gpsimd.tensor_tensor(e2[:], rhi[:], rlo[:], ALU.subtract)
                s = full("s")
                nc.vector.scalar_tensor_tensor(s[:], e2[:], vs0x2_col, e1[:],
                                               ALU.mult, ALU.add)
                t2w = full("t2w")
                nc.gpsimd.tensor_tensor(t2w[:], bi(fvw[:]), bj(hilo[:]),
                                        ALU.mult)
                iedge = epool.tile([128, NCELL], f32, tag="iedge")
                nc.vector.scalar_tensor_tensor(
                    iedge[:].rearrange("p (i j) -> p i j", j=NJ),
                    s[:], col(5), t2w[:], ALU.mult, ALU.add)

                for cch in range(NCHUNK):
                    nc.tensor.matmul(
                        rho_ps[cch][:, h * 256:(h + 1) * 256],
                        iedge[:, cch * 128:(cch + 1) * 128],
                        trw[:, h, :], start=True, stop=True)

            # ---- stage 3, cells-major, per 128-cell chunk ----
            for cch in range(NCHUNK if stages == 'all' else 0):
                rho3 = rho_ps[cch][:].rearrange("p (g n) -> p g n", n=N)
                maxrho = tpool.tile([128, B * K], f32, tag="maxrho")
                nc.vector.reduce_max(maxrho[:], rho3, axis=mybir.AxisListType.X)
                mx_bc = maxrho[:][:, :, None].broadcast_to([128, B * K, N])
                eq = tpool.tile([128, B * K * N], f32, tag="eq")
                nc.vector.tensor_tensor(
                    eq[:].rearrange("p (g n) -> p g n", n=N), rho3, mx_bc,
                    ALU.is_equal)
                cand = tpool.tile([128, B * K * N], f32, tag="cand")
                kb_bc = kbig_bc[:, None, :].broadcast_to([128, B * K, N])
                nc.gpsimd.tensor_tensor(
                    cand[:].rearrange("p (g n) -> p g n", n=N),
                    eq[:].rearrange("p (g n) -> p g n", n=N), kb_bc, ALU.mult)
                cand3 = cand[:].rearrange("p (g n) -> p g n", n=N)
                idxv = tpool.tile([128, B * K], f32, tag="idxv")
                nc.vector.reduce_max(idxv[:], cand3, axis=mybir.AxisListType.X)
                onehot = tpool.tile([128, B * K * N], f32, tag="onehot")
                nc.vector.tensor_tensor(
                    onehot[:].rearrange("p (g n) -> p g n", n=N), cand3,
                    idxv[:][:, :, None].broadcast_to([128, B * K, N]),
                    ALU.is_equal)

                # selection sums via PE: transpose onehot per b, then matmul
                # against w4 -> SEL[cell, (k, {C,sin,cos})]
                oh_t = ppool2.tile([128, 4 * 128], f32, tag="oht")
                sel_ps = ppool2.tile([128, B * K * 3], f32, tag="selps")
                for b in range(B):
                    nc.tensor.transpose(
                        oh_t[:, b * 128:(b + 1) * 128],
                        onehot[:, b * 128:(b + 1) * 128], ident[:])
                ohs = tpool.tile([128, 4 * 128], f32, tag="ohs")
                nc.scalar.copy(ohs[:], oh_t[:])
                for b in range(B):
                    nc.tensor.matmul(
                        sel_ps[:, b * K * 3:(b + 1) * K * 3],
                        ohs[:, b * 128:(b + 1) * 128],
                        tw4[:, b * K * 3:(b + 1) * K * 3],
                        start=True, stop=True)

                sel3 = sel_ps[:].rearrange("p (b k q) -> p b k q", k=K, q=3)
                mx3 = maxrho[:].rearrange("p (b k) -> p b k", k=K)
                intersel = tpool.tile([128, B * K], f32, tag="intersel")
                nc.vector.tensor_tensor(
                    intersel[:].rearrange("p (b k) -> p b k", k=K),
                    mx3, sel3[:, :, :, 0], ALU.mult)
                mask = tpool.tile([128, B * K], f32, tag="mask")
                nc.vector.tensor_single_scalar(mask[:], intersel[:],
                                               halfvol_col, ALU.is_gt)
                outt = opool.tile([128, B, K, 2], f32, tag="outt")
                nc.vector.tensor_tensor(
                    outt[:, :, :, 0], sel3[:, :, :, 1],
                    mask[:].rearrange("p (b k) -> p b k", k=K), ALU.mult)
                nc.vector.tensor_tensor(
                    outt[:, :, :, 1], sel3[:, :, :, 2],
                    mask[:].rearrange("p (b k) -> p b k", k=K), ALU.mult)
                dma_eng = nc.sync if cch % 2 == 0 else nc.scalar
                dma_eng.dma_start(
                    d_out[:, cch * 128 * K:(cch + 1) * 128 * K, :]
                         .rearrange("b (p k) e -> p b k e", k=K),
                    outt[:])

    if stages != 'all':
        with tile.TileContext(nc) as tc2:
            with tc2.tile_pool(name="fin", bufs=1) as fpool:
                z = fpool.tile([128, 64], f32, tag="z")
                nc.gpsimd.memset(z[:], 0.0)
                nc.gpsimd.dma_start(
                    d_out[:, 0:1024, :].rearrange("b (p k) e -> p b k e", k=K),
                    z[:].rearrange("p (b k e) -> p b k e", k=K, e=2))
    nc.compile()
    return nc


def kernel(corners3d, neck_voxel_sizes):
    global _COMPILED
    from concourse.bass_utils import run_bass_kernel_spmd

    cols, rw, w4, ident, consts = _host_prep(corners3d, neck_voxel_sizes)
    if _COMPILED is None:
        _COMPILED = _build()
    nc = _COMPILED
    in_maps = [{"consts": consts[m], "cols": cols, "rw": rw, "w4": w4,
                "ident": ident} for m in range(NCORES)]
    res = run_bass_kernel_spmd(nc, in_maps, list(range(NCORES)))
    out = np.zeros((B, V, 2), np.float32)
    for m in range(NCORES):
        blk = res.results[m]["out"]          # [B, 4096, 2]
        out[:, m * NCELL * K:(m + 1) * NCELL * K, :] = blk
    return out.reshape(B * V, 2)


# revision 33
# speedup vs baseline: 1.3323x; 1.2667x over previous
"""Trainium2 Bass kernel for Box3dEncoder (nn_Box3dEncoder_75453985456565).

Contract: kernel(**inputs) takes the FULL inputs
    corners3d        [4, 16, 8, 3] f32
    neck_voxel_sizes [4, 3]        f32
and returns the FULL output [4*32768, 2] f32.

Strategy (per the voxel-sharding hint): the 64x64x8 grid's first axis (i)
is sharded 8 ways -> each core owns 512 XY cells (8 i-rows x 64 j) x 8 k
levels x 4 batches. Per-box scalars (edge params, z-overlap/volume factors,
heading encodings) are tiny host-side prep, replicated to all cores; all
O(V*N) work runs on-device:

  stage 2 (per core): branch-free polygon-rect clipped area via Green's
    theorem - per edge, integrate clamp(x(t),x0,x1)-x0 over the t-range
    where y(t) is inside [y0,y1], with trapezoid splits at the x-crossings.
    Layout: 128 partitions = (2 batches x 16 boxes x 4 edges), free = 512
    cells. The per-edge factor dy/2 is folded into the PE reduction matmul
    that sums the 4 edges and transposes to cells-major.
  stage 3: rho = area * (zov/C) is a per-(k,n) positive rescale of IoU that
    preserves the argmax (iou = inter/(C-inter) is monotone in inter), so
    argmax over boxes runs division-free; first-occurrence tie-break via
    eq*(BIG-n) + reduce_max; one-hot selection of C, sin, cos; threshold
    mask; interleaved (sin,cos) store.
"""
import numpy as np

B, N, K = 4, 16, 8
CUBE = (64, 64, 8)
LOW = (-32, -32, -4)
NCORES = 8
NI = CUBE[0] // NCORES          # 8 i-rows per core
NJ = CUBE[1]                    # 64
NCELL = NI * NJ                 # 512 cells per core
NCHUNK = NCELL // 128           # 4
V = CUBE[0] * CUBE[1] * CUBE[2]
BIG = 1024.0

_COMPILED = None


def _host_prep(corners3d, neck_voxel_sizes):
    c = np.asarray(corners3d, np.float32)
    vs = np.asarray(neck_voxel_sizes, np.float32)[0]
    vox_vol = np.float32(vs[0]) * np.float32(vs[1]) * np.float32(vs[2])

    poly = c[:, :, :4, :2]                     # [B,N,4,2]
    nxt = np.roll(poly, -1, axis=2)
    xa, ya = poly[..., 0], poly[..., 1]        # [B,N,4]
    xb, yb = nxt[..., 0], nxt[..., 1]
    dx, dy = xb - xa, yb - ya
    with np.errstate(divide='ignore'):
        inv_dx = np.where(np.abs(dx) < 1e-12, np.float32(0),
                          np.float32(1) / np.where(dx == 0, np.float32(1), dx))
        inv_dy = np.where(np.abs(dy) < 1e-12, np.float32(0),
                          np.float32(1) / np.where(dy == 0, np.float32(1), dy))

    zb0 = c[:, :, :, 2].min(axis=2)
    zb1 = c[:, :, :, 2].max(axis=2)
    quad_area = 0.5 * np.abs((xa * yb - xb * ya).sum(axis=2))
    box_vol = quad_area * (zb1 - zb0)
    C = (vox_vol + box_vol + np.float32(1e-9)).astype(np.float32)   # [B,N]
    invC = (np.float32(1) / C).astype(np.float32)

    kk = np.arange(K, dtype=np.float32) + LOW[2]
    z0 = kk * vs[2]
    z1 = (kk + 1) * vs[2]
    zov = np.maximum(np.minimum(z1[None, :, None], zb1[:, None, :])
                     - np.maximum(z0[None, :, None], zb0[:, None, :]),
                     np.float32(0))                                  # [B,K,N]
    # tie-break epsilon: rho_n scaled by (1+eps_n), eps decreasing in n, so a
    # single reduce_max + is_equal yields the first-occurrence argmax; the C
    # selection weight is divided by (1+eps_n) to compensate exactly.
    eps = (np.float32(15) - np.arange(N, dtype=np.float32)) * np.float32(2.0 ** -20)
    zrho = (zov * invC[:, None, :] * (1 + eps)[None, None, :]).astype(np.float32)
    C_w4 = (C / (1 + eps)[None, :]).astype(np.float32)

    d = c[:, :, 0, :2] - c[:, :, 3, :2]
    h = np.sqrt(d[..., 0] ** 2 + d[..., 1] ** 2)
    hs = np.where(h == 0, np.float32(1), h)
    sin = np.where(h > 0, d[..., 1] / hs, np.float32(0)).astype(np.float32)
    cos = np.where(h > 0, d[..., 0] / hs, np.float32(1)).astype(np.float32)

    # --- stage-2 per-partition columns, p = b_lo*64 + n*4 + e, per h-iter ---
    def colpack(a):    # [B,N,4] -> [2h][128]
        return a.reshape(2, 2, N, 4).reshape(2, 128)
    DX_EPS = np.float32(1e-4)
    vert = np.abs(dx) < DX_EPS
    w1 = np.where(vert, np.float32(0), dy * inv_dx * np.float32(0.5))
    w2 = np.where(vert, dy, np.float32(0))
    cols = np.zeros((2, 128, 8), np.float32)
    cols[:, :, 0] = colpack(inv_dy)
    cols[:, :, 1] = colpack(-ya * inv_dy)
    cols[:, :, 2] = colpack((vs[1] - ya) * inv_dy)
    cols[:, :, 3] = colpack(dx)
    cols[:, :, 4] = colpack(xa)
    cols[:, :, 5] = colpack(w1)
    cols[:, :, 6] = colpack(w2)
    cols = np.ascontiguousarray(cols.transpose(1, 0, 2))       # [128,2,8]

    # edge-reduction weights with zrho folded in:
    # rw2[p=(b_lo,n,e), h, (b_lo',k,n')] = (b_lo'==b_lo & n'==n) * dy/2 * zrho[b,k,n]
    rw = np.zeros((128, 2, 2, K, N), np.float32)
    for h in range(2):
        for p in range(128):
            b_lo, n = p // 64, (p % 64) // 4
            rw[p, h, b_lo, :, n] = zrho[2 * h + b_lo, :, n]
    rw = np.ascontiguousarray(rw.reshape(128, 2, 2 * K * N))   # [128,2,256]

    # selection matmul weights: w4[b][(k,n), (k',q)] = (k==k') * w_q[b,n]
    # q in {C, sin, cos}
    w4 = np.zeros((128, B, K, 3), np.float32)
    for p in range(128):
        kq, n = p // N, p % N
        w4[p, :, kq, 0] = C_w4[:, n]
        w4[p, :, kq, 1] = sin[:, n]
        w4[p, :, kq, 2] = cos[:, n]
    w4 = np.ascontiguousarray(w4.reshape(128, B * K * 3))      # [128,96]

    # --- cells-major broadcast constants (partition-replicated by host) ---
    kbig = BIG - np.arange(N, dtype=np.float32)                # [16]
    jj = np.arange(NJ, dtype=np.float32) + LOW[1]
    y0 = (jj * vs[1]).astype(np.float32)                       # [64]
    halfvol = np.float32(0.5) * vox_vol

    consts = []
    for m in range(NCORES):
        ii = np.arange(NI, dtype=np.float32) + (m * NI + LOW[0])
        x0 = (ii * vs[0]).astype(np.float32)                   # [8]
        row = np.concatenate([kbig, y0, x0,
                              [halfvol, vs[0], -vs[0], 2 * vs[0]]]).astype(np.float32)
        consts.append(np.broadcast_to(row, (128, row.size)).copy())
    ident = np.eye(128, dtype=np.float32)
    return cols, rw, w4, ident, consts


def _build(stages='all'):
    import concourse.bass as bass
    import concourse.tile as tile
    from concourse import bacc, mybir

    f32 = mybir.dt.float32
    ALU = mybir.AluOpType
    ACT = mybir.ActivationFunctionType

    CW = 16 + 64 + 8 + 4
    OFF_KBIG, OFF_Y0, OFF_X0, OFF_MISC = 0, 16, 80, 88

    nc = bacc.Bacc("TRN2", target_bir_lowering=False, debug=False,
                   num_devices=NCORES)
    d_consts = nc.dram_tensor("consts", [128, CW], f32, kind="ExternalInput")
    d_cols = nc.dram_tensor("cols", [128, 2, 8], f32, kind="ExternalInput")
    d_rw = nc.dram_tensor("rw", [128, 2, 256], f32, kind="ExternalInput")
    d_w4 = nc.dram_tensor("w4", [128, B * K * 3], f32, kind="ExternalInput")
    d_ident = nc.dram_tensor("ident", [128, 128], f32, kind="ExternalInput")
    d_out = nc.dram_tensor("out", [B, NCELL * K, 2], f32, kind="ExternalOutput")

    with tile.TileContext(nc) as tc:
        with (
            tc.tile_pool(name="const", bufs=1) as cpool,
            tc.tile_pool(name="small", bufs=4) as spool,
            tc.tile_pool(name="work", bufs=6) as wpool,
            tc.tile_pool(name="edge", bufs=4) as epool,
            tc.tile_pool(name="st3", bufs=6) as tpool,
            tc.tile_pool(name="outp", bufs=4) as opool,
            tc.tile_pool(name="psum", bufs=1, space=bass.MemorySpace.PSUM) as ppool,
            tc.tile_pool(name="psum2", bufs=2, space=bass.MemorySpace.PSUM) as ppool2,
        ):
            tco = cpool.tile([128, CW], f32, tag="consts")
            nc.sync.dma_start(tco[:], d_consts[:])
            tcols = cpool.tile([128, 2, 8], f32, tag="cols")
            nc.scalar.dma_start(tcols[:], d_cols[:])
            trw = cpool.tile([128, 2, 256], f32, tag="rw")
            nc.scalar.dma_start(trw[:], d_rw[:])
            tw4 = cpool.tile([128, B * K * 3], f32, tag="w4")
            nc.sync.dma_start(tw4[:], d_w4[:])
            ident = cpool.tile([128, 128], f32, tag="ident")
            nc.sync.dma_start(ident[:], d_ident[:])

            kbig_bc = tco[:, OFF_KBIG:OFF_KBIG + 16]
            y0_bc = tco[:, OFF_Y0:OFF_Y0 + 64]
            x0_bc = tco[:, OFF_X0:OFF_X0 + 8]
            halfvol_col = tco[:, OFF_MISC:OFF_MISC + 1]
            vs0_col = tco[:, OFF_MISC + 1:OFF_MISC + 2]
            nvs0_col = tco[:, OFF_MISC + 2:OFF_MISC + 3]
            vs0x2_col = tco[:, OFF_MISC + 3:OFF_MISC + 4]

            def bj(ap):   # [128,64] j-tile -> broadcast over i: [128,8,64]
                return ap[:, None, :].broadcast_to([128, NI, NJ])

            def bi(ap):   # [128,8] i-tile -> broadcast over j: [128,8,64]
                return ap[:, :, None].broadcast_to([128, NI, NJ])

            rho_ps = []
            for c in range(NCHUNK):
                rp = ppool.tile([128, B * K * N], f32, tag=f"rho{c}")
                rho_ps.append(rp)
            for h in range(2 if stages != 'none' else 0):
                col = lambda q: tcols[:, h, q:q + 1]
                # j-only quantities [128, 64]
                ty0 = spool.tile([128, NJ], f32, tag="ty0")
                nc.scalar.activation(ty0[:], y0_bc, ACT.Identity,
                                     bias=col(1), scale=col(0))
                ty1 = spool.tile([128, NJ], f32, tag="ty1")
                nc.scalar.activation(ty1[:], y0_bc, ACT.Identity,
                                     bias=col(2), scale=col(0))
                u0 = spool.tile([128, NJ], f32, tag="u0")
                nc.vector.tensor_scalar(u0[:], ty0[:], 0.0, 1.0, ALU.max, ALU.min)
                u1 = spool.tile([128, NJ], f32, tag="u1")
                nc.vector.tensor_scalar(u1[:], ty1[:], 0.0, 1.0, ALU.max, ALU.min)
                lo = spool.tile([128, NJ], f32, tag="lo")
                nc.vector.tensor_tensor(lo[:], u0[:], u1[:], ALU.min)
                hi = spool.tile([128, NJ], f32, tag="hi")
                nc.vector.tensor_tensor(hi[:], u0[:], u1[:], ALU.max)
                hilo = spool.tile([128, NJ], f32, tag="hilo")
                nc.vector.tensor_tensor(hilo[:], hi[:], lo[:], ALU.subtract)
                # i-only quantities [128, 8]
                x0mxa = spool.tile([128, NI], f32, tag="x0mxa")
                nc.vector.tensor_single_scalar(x0mxa[:], x0_bc, col(4),
                                               ALU.subtract)
                # vertical-edge fallback: Fv = clamp(xa - x0, 0, vs0), * w2
                fvw = spool.tile([128, NI], f32, tag="fvw")
                nc.vector.tensor_scalar(fvw[:], x0mxa[:], -1.0, 0.0,
                                        ALU.mult, ALU.max)
                nc.vector.tensor_single_scalar(fvw[:], fvw[:], vs0_col, ALU.min)
                nc.vector.tensor_single_scalar(fvw[:], fvw[:], col(6), ALU.mult)

                NIH = NI // 2
                for half in range(2):
                    isl = slice(half * NIH, (half + 1) * NIH)

                    def full(tag):
                        t = wpool.tile([128, NIH, NJ], f32, tag=tag)
                        return t

                    def bjh(ap):
                        return ap[:, None, :].broadcast_to([128, NIH, NJ])

                    def bih(ap):
                        return ap[:, isl, None].broadcast_to([128, NIH, NJ])

                    # g at t=lo and t=hi:  g = dx*t - (x0 - xa)
                    glo = full("glo")
                    nc.vector.scalar_tensor_tensor(glo[:], bjh(lo[:]), col(3),
                                                   bih(x0mxa[:]), ALU.mult,
                                                   ALU.subtract)
                    ghi = full("ghi")
                    nc.vector.scalar_tensor_tensor(ghi[:], bjh(hi[:]), col(3),
                                                   bih(x0mxa[:]), ALU.mult,
                                                   ALU.subtract)
                    # H(u) = 0.5*clamp(u,0,c)^2 + c*relu(u-c); w1 carries 0.5
                    clo = full("clo")
                    nc.vector.tensor_scalar(clo[:], glo[:], 0.0, vs0_col,
                                            ALU.max, ALU.min)
                    chi = full("chi")
                    nc.gpsimd.tensor_scalar(chi[:], ghi[:], 0.0, vs0_col,
                                            ALU.max, ALU.min)
                    sqlo = full("sqlo")
                    nc.scalar.activation(sqlo[:], clo[:], ACT.Square)
                    sqhi = full("sqhi")
                    nc.scalar.activation(sqhi[:], chi[:], ACT.Square)
                    rlo = full("rlo")
                    nc.scalar.activation(rlo[:], glo[:], ACT.Relu,
                                         bias=nvs0_col)
                    rhi = full("rhi")
                    nc.scalar.activation(rhi[:], ghi[:], ACT.Relu,
                                         bias=nvs0_col)
                    e1 = full("e1")
                    nc.vector.tensor_tensor(e1[:], sqhi[:], sqlo[:],
                                            ALU.subtract)
                    e2 = full("e2")
                    nc.gpsimd.tensor_tensor(e2[:], rhi[:], rlo[:],
                                            ALU.subtract)
                    s = full("s")
                    nc.vector.scalar_tensor_tensor(s[:], e2[:], vs0x2_col,
                                                   e1[:], ALU.mult, ALU.add)
                    t2w = full("t2w")
                    nc.gpsimd.tensor_tensor(t2w[:], bih(fvw[:]), bjh(hilo[:]),
                                            ALU.mult)
                    iedge = epool.tile([128, NCELL // 2], f32, tag="iedge")
                    nc.vector.scalar_tensor_tensor(
                        iedge[:].rearrange("p (i j) -> p i j", j=NJ),
                        s[:], col(5), t2w[:], ALU.mult, ALU.add)

                    for cc in range(2):
                        cch = half * 2 + cc
                        nc.tensor.matmul(
                            rho_ps[cch][:, h * 256:(h + 1) * 256],
                            iedge[:, cc * 128:(cc + 1) * 128],
                            trw[:, h, :], start=True, stop=True)

            # ---- stage 3, cells-major, per 128-cell chunk ----
            for cch in range(NCHUNK if stages == 'all' else 0):
                rho3 = rho_ps[cch][:].rearrange("p (g n) -> p g n", n=N)
                maxrho = tpool.tile([128, B * K], f32, tag="maxrho")
                nc.vector.reduce_max(maxrho[:], rho3, axis=mybir.AxisListType.X)
                mx_bc = maxrho[:][:, :, None].broadcast_to([128, B * K, N])
                onehot = tpool.tile([128, B * K * N], f32, tag="onehot")
                nc.vector.tensor_tensor(
                    onehot[:].rearrange("p (g n) -> p g n", n=N), rho3, mx_bc,
                    ALU.is_equal)

                # selection sums via PE: transpose onehot per b, then matmul
                # against w4 -> SEL[cell, (k, {C,sin,cos})]
                oh_t = ppool2.tile([128, 4 * 128], f32, tag="oht")
                sel_ps = ppool2.tile([128, B * K * 3], f32, tag="selps")
                for b in range(B):
                    nc.tensor.transpose(
                        oh_t[:, b * 128:(b + 1) * 128],
                        onehot[:, b * 128:(b + 1) * 128], ident[:])
                ohs = tpool.tile([128, 4 * 128], f32, tag="ohs")
                nc.scalar.copy(ohs[:], oh_t[:])
                for b in range(B):
                    nc.tensor.matmul(
                        sel_ps[:, b * K * 3:(b + 1) * K * 3],
                        ohs[:, b * 128:(b + 1) * 128],
                        tw4[:, b * K * 3:(b + 1) * K * 3],
                        start=True, stop=True)

                sel3 = sel_ps[:].rearrange("p (b k q) -> p b k q", k=K, q=3)
                mx3 = maxrho[:].rearrange("p (b k) -> p b k", k=K)
                intersel = tpool.tile([128, B * K], f32, tag="intersel")
                nc.vector.tensor_tensor(
                    intersel[:].rearrange("p (b k) -> p b k", k=K),
                    mx3, sel3[:, :, :, 0], ALU.mult)
                mask = tpool.tile([128, B * K], f32, tag="mask")
                nc.vector.tensor_single_scalar(mask[:], intersel[:],
                                               halfvol_col, ALU.is_gt)
                outt = opool.tile([128, B, K, 2], f32, tag="outt")
                nc.vector.tensor_tensor(
                    outt[:, :, :, 0], sel3[:, :, :, 1],
                    mask[:].rearrange("p (b k) -> p b k", k=K), ALU.mult)
                nc.vector.tensor_tensor(
                    outt[:, :, :, 1], sel3[:, :, :, 2],
                    mask[:].rearrange("p (b k) -> p b k", k=K), ALU.mult)
                dma_eng = nc.sync if cch % 2 == 0 else nc.scalar
                dma_eng.dma_start(
                    d_out[:, cch * 128 * K:(cch + 1) * 128 * K, :]
                         .rearrange("b (p k) e -> p b k e", k=K),
                    outt[:])

    if stages != 'all':
        with tile.TileContext(nc) as tc2:
            with tc2.tile_pool(name="fin", bufs=1) as fpool:
                z = fpool.tile([128, 64], f32, tag="z")
                nc.gpsimd.memset(z[:], 0.0)
                nc.gpsimd.dma_start(
                    d_out[:, 0:1024, :].rearrange("b (p k) e -> p b k e", k=K),
                    z[:].rearrange("p (b k e) -> p b k e", k=K, e=2))
    nc.compile()
    return nc


def kernel(corners3d, neck_voxel_sizes):
    global _COMPILED
    from concourse.bass_utils import run_bass_kernel_spmd

    cols, rw, w4, ident, consts = _host_prep(corners3d, neck_voxel_sizes)
    if _COMPILED is None:
        _COMPILED = _build()
    nc = _COMPILED
    in_maps = [{"consts": consts[m], "cols": cols, "rw": rw, "w4": w4,
                "ident": ident} for m in range(NCORES)]
    res = run_bass_kernel_spmd(nc, in_maps, list(range(NCORES)))
    out = np.zeros((B, V, 2), np.float32)
    for m in range(NCORES):
        blk = res.results[m]["out"]          # [B, 4096, 2]
        out[:, m * NCELL * K:(m + 1) * NCELL * K, :] = blk
    return out.reshape(B * V, 2)


# revision 40
# speedup vs baseline: 1.3630x; 1.0231x over previous
"""Trainium2 Bass kernel for Box3dEncoder (nn_Box3dEncoder_75453985456565).

Contract: kernel(**inputs) takes the FULL inputs
    corners3d        [4, 16, 8, 3] f32
    neck_voxel_sizes [4, 3]        f32
and returns the FULL output [4*32768, 2] f32.

Strategy (per the voxel-sharding hint): the 64x64x8 grid's first axis (i)
is sharded 8 ways -> each core owns 512 XY cells (8 i-rows x 64 j) x 8 k
levels x 4 batches. Per-box scalars (edge params, z-overlap/volume factors,
heading encodings) are tiny host-side prep, replicated to all cores; all
O(V*N) work runs on-device:

  stage 2 (per core): branch-free polygon-rect clipped area via Green's
    theorem - per edge, integrate clamp(x(t),x0,x1)-x0 over the t-range
    where y(t) is inside [y0,y1], with trapezoid splits at the x-crossings.
    Layout: 128 partitions = (2 batches x 16 boxes x 4 edges), free = 512
    cells. The per-edge factor dy/2 is folded into the PE reduction matmul
    that sums the 4 edges and transposes to cells-major.
  stage 3: rho = area * (zov/C) is a per-(k,n) positive rescale of IoU that
    preserves the argmax (iou = inter/(C-inter) is monotone in inter), so
    argmax over boxes runs division-free; first-occurrence tie-break via
    eq*(BIG-n) + reduce_max; one-hot selection of C, sin, cos; threshold
    mask; interleaved (sin,cos) store.
"""
import numpy as np

B, N, K = 4, 16, 8
CUBE = (64, 64, 8)
LOW = (-32, -32, -4)
NCORES = 8
NI = CUBE[0] // NCORES          # 8 i-rows per core
NJ = CUBE[1]                    # 64
NCELL = NI * NJ                 # 512 cells per core
NCHUNK = NCELL // 128           # 4
V = CUBE[0] * CUBE[1] * CUBE[2]
BIG = 1024.0

_COMPILED = None


def _host_prep(corners3d, neck_voxel_sizes):
    c = np.asarray(corners3d, np.float32)
    vs = np.asarray(neck_voxel_sizes, np.float32)[0]
    vox_vol = np.float32(vs[0]) * np.float32(vs[1]) * np.float32(vs[2])

    poly = c[:, :, :4, :2]                     # [B,N,4,2]
    nxt = np.roll(poly, -1, axis=2)
    xa, ya = poly[..., 0], poly[..., 1]        # [B,N,4]
    xb, yb = nxt[..., 0], nxt[..., 1]
    dx, dy = xb - xa, yb - ya
    with np.errstate(divide='ignore'):
        inv_dx = np.where(np.abs(dx) < 1e-12, np.float32(0),
                          np.float32(1) / np.where(dx == 0, np.float32(1), dx))
        inv_dy = np.where(np.abs(dy) < 1e-12, np.float32(0),
                          np.float32(1) / np.where(dy == 0, np.float32(1), dy))

    zb0 = c[:, :, :, 2].min(axis=2)
    zb1 = c[:, :, :, 2].max(axis=2)
    quad_area = 0.5 * np.abs((xa * yb - xb * ya).sum(axis=2))
    box_vol = quad_area * (zb1 - zb0)
    C = (vox_vol + box_vol + np.float32(1e-9)).astype(np.float32)   # [B,N]
    invC = (np.float32(1) / C).astype(np.float32)

    kk = np.arange(K, dtype=np.float32) + LOW[2]
    z0 = kk * vs[2]
    z1 = (kk + 1) * vs[2]
    zov = np.maximum(np.minimum(z1[None, :, None], zb1[:, None, :])
                     - np.maximum(z0[None, :, None], zb0[:, None, :]),
                     np.float32(0))                                  # [B,K,N]
    # tie-break epsilon: rho_n scaled by (1+eps_n), eps decreasing in n, so a
    # single reduce_max + is_equal yields the first-occurrence argmax; the C
    # selection weight is divided by (1+eps_n) to compensate exactly.
    eps = (np.float32(15) - np.arange(N, dtype=np.float32)) * np.float32(2.0 ** -20)
    zrho = (zov * invC[:, None, :] * (1 + eps)[None, None, :]).astype(np.float32)
    C_w4 = (C / (1 + eps)[None, :]).astype(np.float32)

    d = c[:, :, 0, :2] - c[:, :, 3, :2]
    h = np.sqrt(d[..., 0] ** 2 + d[..., 1] ** 2)
    hs = np.where(h == 0, np.float32(1), h)
    sin = np.where(h > 0, d[..., 1] / hs, np.float32(0)).astype(np.float32)
    cos = np.where(h > 0, d[..., 0] / hs, np.float32(1)).astype(np.float32)

    # --- stage-2 per-partition columns, p = b_lo*64 + n*4 + e, per h-iter ---
    def colpack(a):    # [B,N,4] -> [2h][128]
        return a.reshape(2, 2, N, 4).reshape(2, 128)
    DX_EPS = np.float32(1e-4)
    vert = np.abs(dx) < DX_EPS
    w1 = np.where(vert, np.float32(0), dy * inv_dx * np.float32(0.5))
    w2 = np.where(vert, dy, np.float32(0))
    cols = np.zeros((2, 128, 8), np.float32)
    cols[:, :, 0] = colpack(inv_dy)
    cols[:, :, 1] = colpack(-ya * inv_dy)
    cols[:, :, 2] = colpack((vs[1] - ya) * inv_dy)
    cols[:, :, 3] = colpack(dx)
    cols[:, :, 4] = colpack(xa)
    cols[:, :, 5] = colpack(w1)
    cols[:, :, 6] = colpack(w2)
    cols = np.ascontiguousarray(cols.transpose(1, 0, 2))       # [128,2,8]

    # edge-reduction weights with zrho folded in:
    # rw2[p=(b_lo,n,e), h, (b_lo',k,n')] = (b_lo'==b_lo & n'==n) * dy/2 * zrho[b,k,n]
    rw = np.zeros((128, 2, 2, K, N), np.float32)
    for h in range(2):
        for p in range(128):
            b_lo, n = p // 64, (p % 64) // 4
            rw[p, h, b_lo, :, n] = zrho[2 * h + b_lo, :, n]
    rw = np.ascontiguousarray(rw.reshape(128, 2, 2 * K * N))   # [128,2,256]

    # selection matmul weights: w4[b][(k,n), (k',q)] = (k==k') * w_q[b,n]
    # q in {C, sin, cos}
    w4 = np.zeros((128, B, K, 3), np.float32)
    for p in range(128):
        kq, n = p // N, p % N
        w4[p, :, kq, 0] = C_w4[:, n]
        w4[p, :, kq, 1] = sin[:, n]
        w4[p, :, kq, 2] = cos[:, n]
    w4 = np.ascontiguousarray(w4.reshape(128, B * K * 3))      # [128,96]

    # --- cells-major broadcast constants (partition-replicated by host) ---
    kbig = BIG - np.arange(N, dtype=np.float32)                # [16]
    jj = np.arange(NJ, dtype=np.float32) + LOW[1]
    y0 = (jj * vs[1]).astype(np.float32)                       # [64]
    halfvol = np.float32(0.5) * vox_vol

    consts = []
    for m in range(NCORES):
        ii = np.arange(NI, dtype=np.float32) + (m * NI + LOW[0])
        x0 = (ii * vs[0]).astype(np.float32)                   # [8]
        row = np.concatenate([kbig, y0, x0,
                              [halfvol, vs[0], -vs[0], 2 * vs[0]]]).astype(np.float32)
        cc = np.concatenate(
            [np.broadcast_to(row, (128, row.size)), cols.reshape(128, 16)],
            axis=1).astype(np.float32)
        consts.append(np.ascontiguousarray(cc))
    ident = np.eye(128, dtype=np.float32)
    return rw, w4, ident, consts


def _build(stages='all'):
    import concourse.bass as bass
    import concourse.tile as tile
    from concourse import bacc, mybir

    f32 = mybir.dt.float32
    ALU = mybir.AluOpType
    ACT = mybir.ActivationFunctionType

    CW = 16 + 64 + 8 + 4 + 16
    OFF_KBIG, OFF_Y0, OFF_X0, OFF_MISC, OFF_COLS = 0, 16, 80, 88, 92

    nc = bacc.Bacc("TRN2", target_bir_lowering=False, debug=False,
                   num_devices=NCORES)
    d_consts = nc.dram_tensor("consts", [128, CW], f32, kind="ExternalInput")
    d_rw = nc.dram_tensor("rw", [128, 2, 256], f32, kind="ExternalInput")
    d_w4 = nc.dram_tensor("w4", [128, B * K * 3], f32, kind="ExternalInput")
    d_ident = nc.dram_tensor("ident", [128, 128], f32, kind="ExternalInput")
    d_out = nc.dram_tensor("out", [B, NCELL * K, 2], f32, kind="ExternalOutput")

    with tile.TileContext(nc) as tc:
        with (
            tc.tile_pool(name="const", bufs=1) as cpool,
            tc.tile_pool(name="small", bufs=4) as spool,
            tc.tile_pool(name="work", bufs=6) as wpool,
            tc.tile_pool(name="edge", bufs=4) as epool,
            tc.tile_pool(name="st3", bufs=6) as tpool,
            tc.tile_pool(name="outp", bufs=4) as opool,
            tc.tile_pool(name="psum", bufs=1, space=bass.MemorySpace.PSUM) as ppool,
            tc.tile_pool(name="psum2", bufs=2, space=bass.MemorySpace.PSUM) as ppool2,
        ):
            tco = cpool.tile([128, CW], f32, tag="consts")
            nc.sync.dma_start(tco[:], d_consts[:])
            trw = cpool.tile([128, 2, 256], f32, tag="rw")
            nc.scalar.dma_start(trw[:], d_rw[:])
            tw4 = cpool.tile([128, B * K * 3], f32, tag="w4")
            nc.sync.dma_start(tw4[:], d_w4[:])
            ident = cpool.tile([128, 128], f32, tag="ident")
            nc.sync.dma_start(ident[:], d_ident[:])

            kbig_bc = tco[:, OFF_KBIG:OFF_KBIG + 16]
            y0_bc = tco[:, OFF_Y0:OFF_Y0 + 64]
            x0_bc = tco[:, OFF_X0:OFF_X0 + 8]
            halfvol_col = tco[:, OFF_MISC:OFF_MISC + 1]
            vs0_col = tco[:, OFF_MISC + 1:OFF_MISC + 2]
            nvs0_col = tco[:, OFF_MISC + 2:OFF_MISC + 3]
            vs0x2_col = tco[:, OFF_MISC + 3:OFF_MISC + 4]

            def bj(ap):   # [128,64] j-tile -> broadcast over i: [128,8,64]
                return ap[:, None, :].broadcast_to([128, NI, NJ])

            def bi(ap):   # [128,8] i-tile -> broadcast over j: [128,8,64]
                return ap[:, :, None].broadcast_to([128, NI, NJ])

            rho_ps = []
            for c in range(NCHUNK):
                rp = ppool.tile([128, B * K * N], f32, tag=f"rho{c}")
                rho_ps.append(rp)
            for h in range(2 if stages != 'none' else 0):
                col = lambda q: tco[:, OFF_COLS + h * 8 + q:OFF_COLS + h * 8 + q + 1]
                # j-only quantities [128, 64]
                ty0 = spool.tile([128, NJ], f32, tag="ty0")
                nc.scalar.activation(ty0[:], y0_bc, ACT.Identity,
                                     bias=col(1), scale=col(0))
                ty1 = spool.tile([128, NJ], f32, tag="ty1")
                nc.scalar.activation(ty1[:], y0_bc, ACT.Identity,
                                     bias=col(2), scale=col(0))
                u0 = spool.tile([128, NJ], f32, tag="u0")
                nc.vector.tensor_scalar(u0[:], ty0[:], 0.0, 1.0, ALU.max, ALU.min)
                u1 = spool.tile([128, NJ], f32, tag="u1")
                nc.vector.tensor_scalar(u1[:], ty1[:], 0.0, 1.0, ALU.max, ALU.min)
                lo = spool.tile([128, NJ], f32, tag="lo")
                nc.vector.tensor_tensor(lo[:], u0[:], u1[:], ALU.min)
                hi = spool.tile([128, NJ], f32, tag="hi")
                nc.vector.tensor_tensor(hi[:], u0[:], u1[:], ALU.max)
                hilo = spool.tile([128, NJ], f32, tag="hilo")
                nc.vector.tensor_tensor(hilo[:], hi[:], lo[:], ALU.subtract)
                # i-only quantities [128, 8]
                x0mxa = spool.tile([128, NI], f32, tag="x0mxa")
                nc.vector.tensor_single_scalar(x0mxa[:], x0_bc, col(4),
                                               ALU.subtract)
                # vertical-edge fallback: Fv = clamp(xa - x0, 0, vs0), * w2
                fvw = spool.tile([128, NI], f32, tag="fvw")
                nc.vector.tensor_scalar(fvw[:], x0mxa[:], -1.0, 0.0,
                                        ALU.mult, ALU.max)
                nc.vector.tensor_single_scalar(fvw[:], fvw[:], vs0_col, ALU.min)
                nc.vector.tensor_single_scalar(fvw[:], fvw[:], col(6), ALU.mult)

                NIH = NI // 2
                for half in range(2):
                    isl = slice(half * NIH, (half + 1) * NIH)

                    def full(tag):
                        t = wpool.tile([128, NIH, NJ], f32, tag=tag)
                        return t

                    def bjh(ap):
                        return ap[:, None, :].broadcast_to([128, NIH, NJ])

                    def bih(ap):
                        return ap[:, isl, None].broadcast_to([128, NIH, NJ])

                    # g at t=lo and t=hi:  g = dx*t - (x0 - xa)
                    glo = full("glo")
                    nc.vector.scalar_tensor_tensor(glo[:], bjh(lo[:]), col(3),
                                                   bih(x0mxa[:]), ALU.mult,
                                                   ALU.subtract)
                    ghi = full("ghi")
                    nc.vector.scalar_tensor_tensor(ghi[:], bjh(hi[:]), col(3),
                                                   bih(x0mxa[:]), ALU.mult,
                                                   ALU.subtract)
                    # H(u) = 0.5*clamp(u,0,c)^2 + c*relu(u-c); w1 carries 0.5
                    clo = full("clo")
                    nc.vector.tensor_scalar(clo[:], glo[:], 0.0, vs0_col,
                                            ALU.max, ALU.min)
                    chi = full("chi")
                    nc.gpsimd.tensor_scalar(chi[:], ghi[:], 0.0, vs0_col,
                                            ALU.max, ALU.min)
                    sqlo = full("sqlo")
                    nc.scalar.activation(sqlo[:], clo[:], ACT.Square)
                    sqhi = full("sqhi")
                    nc.scalar.activation(sqhi[:], chi[:], ACT.Square)
                    rlo = full("rlo")
                    nc.scalar.activation(rlo[:], glo[:], ACT.Relu,
                                         bias=nvs0_col)
                    rhi = full("rhi")
                    nc.scalar.activation(rhi[:], ghi[:], ACT.Relu,
                                         bias=nvs0_col)
                    e1 = full("e1")
                    nc.vector.tensor_tensor(e1[:], sqhi[:], sqlo[:],
                                            ALU.subtract)
                    e2 = full("e2")
                    nc.gpsimd.tensor_tensor(e2[:], rhi[:], rlo[:],
                                            ALU.subtract)
                    s = full("s")
                    nc.vector.scalar_tensor_tensor(s[:], e2[:], vs0x2_col,
                                                   e1[:], ALU.mult, ALU.add)
                    t2w = full("t2w")
                    nc.gpsimd.tensor_tensor(t2w[:], bih(fvw[:]), bjh(hilo[:]),
                                            ALU.mult)
                    iedge = epool.tile([128, NCELL // 2], f32, tag="iedge")
                    nc.vector.scalar_tensor_tensor(
                        iedge[:].rearrange("p (i j) -> p i j", j=NJ),
                        s[:], col(5), t2w[:], ALU.mult, ALU.add)

                    for cc in range(2):
                        cch = half * 2 + cc
                        nc.tensor.matmul(
                            rho_ps[cch][:, h * 256:(h + 1) * 256],
                            iedge[:, cc * 128:(cc + 1) * 128],
                            trw[:, h, :], start=True, stop=True)

            # ---- stage 3, cells-major, per 128-cell chunk ----
            for cch in range(NCHUNK if stages == 'all' else 0):
                rho3 = rho_ps[cch][:].rearrange("p (g n) -> p g n", n=N)
                maxrho = tpool.tile([128, B * K], f32, tag="maxrho")
                nc.vector.reduce_max(maxrho[:], rho3, axis=mybir.AxisListType.X)
                mx_bc = maxrho[:][:, :, None].broadcast_to([128, B * K, N])
                onehot = tpool.tile([128, B * K * N], f32, tag="onehot")
                nc.vector.tensor_tensor(
                    onehot[:].rearrange("p (g n) -> p g n", n=N), rho3, mx_bc,
                    ALU.is_equal)

                # selection sums via PE: transpose onehot per b, then matmul
                # against w4 -> SEL[cell, (k, {C,sin,cos})]
                oh_t = ppool2.tile([128, 4 * 128], f32, tag="oht")
                sel_ps = ppool2.tile([128, B * K * 3], f32, tag="selps")
                for b in range(B):
                    nc.tensor.transpose(
                        oh_t[:, b * 128:(b + 1) * 128],
                        onehot[:, b * 128:(b + 1) * 128], ident[:])
                ohs = tpool.tile([128, 4 * 128], f32, tag="ohs")
                nc.scalar.copy(ohs[:], oh_t[:])
                for b in range(B):
                    nc.tensor.matmul(
                        sel_ps[:, b * K * 3:(b + 1) * K * 3],
                        ohs[:, b * 128:(b + 1) * 128],
                        tw4[:, b * K * 3:(b + 1) * K * 3],
                        start=True, stop=True)

                sel3 = sel_ps[:].rearrange("p (b k q) -> p b k q", k=K, q=3)
                mx3 = maxrho[:].rearrange("p (b k) -> p b k", k=K)
                intersel = tpool.tile([128, B * K], f32, tag="intersel")
                nc.vector.tensor_tensor(
                    intersel[:].rearrange("p (b k) -> p b k", k=K),
                    mx3, sel3[:, :, :, 0], ALU.mult)
                mask = tpool.tile([128, B * K], f32, tag="mask")
                nc.vector.tensor_single_scalar(mask[:], intersel[:],
                                               halfvol_col, ALU.is_gt)
                outt = opool.tile([128, B, K, 2], f32, tag="outt")
                nc.vector.tensor_tensor(
                    outt[:, :, :, 0], sel3[:, :, :, 1],
                    mask[:].rearrange("p (b k) -> p b k", k=K), ALU.mult)
                nc.vector.tensor_tensor(
                    outt[:, :, :, 1], sel3[:, :, :, 2],
                    mask[:].rearrange("p (b k) -> p b k", k=K), ALU.mult)
                dma_eng = nc.sync if cch % 2 == 0 else nc.scalar
                dma_eng.dma_start(
                    d_out[:, cch * 128 * K:(cch + 1) * 128 * K, :]
                         .rearrange("b (p k) e -> p b k e", k=K),
                    outt[:])

    if stages != 'all':
        with tile.TileContext(nc) as tc2:
            with tc2.tile_pool(name="fin", bufs=1) as fpool:
                z = fpool.tile([128, 64], f32, tag="z")
                nc.gpsimd.memset(z[:], 0.0)
                nc.gpsimd.dma_start(
                    d_out[:, 0:1024, :].rearrange("b (p k) e -> p b k e", k=K),
                    z[:].rearrange("p (b k e) -> p b k e", k=K, e=2))
    nc.compile()
    return nc


def kernel(corners3d, neck_voxel_sizes):
    global _COMPILED
    from concourse.bass_utils import run_bass_kernel_spmd

    rw, w4, ident, consts = _host_prep(corners3d, neck_voxel_sizes)
    if _COMPILED is None:
        _COMPILED = _build()
    nc = _COMPILED
    in_maps = [{"consts": consts[m], "rw": rw, "w4": w4,
                "ident": ident} for m in range(NCORES)]
    res = run_bass_kernel_spmd(nc, in_maps, list(range(NCORES)))
    out = np.zeros((B, V, 2), np.float32)
    for m in range(NCORES):
        blk = res.results[m]["out"]          # [B, 4096, 2]
        out[:, m * NCELL * K:(m + 1) * NCELL * K, :] = blk
    return out.reshape(B * V, 2)
